# revision 9
# baseline (speedup 1.0000x reference)
"""Trainium2 Bass kernel for blocked-DCT high-frequency extractor.

Computes, for x (64, 3, 512, 512) f32:
  gray = 0.299*R + 0.587*G + 0.114*B                     (B,1,H,W)
  per 8x8 block:  Y = mask * (D @ block @ D.T)           (2D DCT + high-pass)
  output (64, 1, 512, 512) f32

Strategy (pure data parallel over batch, 8 batches/core on 8 cores; the
kernel is HBM-bound: 24 MiB in + 8 MiB out per core => ~94 us floor at
the ~358 GB/s per-core HBM limit).

Per core, per (batch, 128-row chunk) of the image:
  1. One fused 768 KB DMA on the SP HWDGE queue brings all 3 channel
     chunks into a (128h, 3*512w) tile (2 KB contiguous runs).
  2. Grayscale spread over three engines so none saturates:
     g0 = x0*(w0/w2) + x2 on DVE (scalar_tensor_tensor),
     gs = x1*(w1/w2) on ACT, g1 = g0 + gs on GpSimd.
  3. H-direction DCT: one matmul with sqrt(w2) * (I_16 kron D^T).
     Both matmuls run with operands bitcast to float32r: the PE
     processes f32r at 1 cycle/row (vs 4 for plain f32), and the
     precision loss is well inside the 2e-2 gate.
  4. DVE stream-transpose (independent 32x32 blocks) read straight from
     PSUM. Because 8 | 32, this puts w%32 (which contains the
     intra-block w index b) on partitions.
  5. W-direction DCT: one matmul with the same stationary weight
     (the two sqrt(w2) factors give the grayscale w2 scale in total).
  6. High-pass mask on ACT as two strided PSUM->SBUF copies: columns
     with u<4 are scaled by a per-partition 0/1 vector (zero iff v<4),
     u>=4 columns are a plain copy. This keeps the mask off the DVE,
     whose two structural transposes are the tightest compute budget.
  7. DVE stream-transpose back -> exact (hfreq, wfreq) output layout.
  8. 256 KB contiguous output DMA on the ACT HWDGE queue (separate
     queue from the input stream).

The 32x32 block transpose is an involution whose block-nesting (8 | 32)
makes both DCT matmuls use the same I_16 kron D^T stationary weight and
lands the final result in natural row-major layout with zero TensorE
transposes.
"""

import os

import numpy as np

import concourse.bacc as bacc
import concourse.mybir as mybir
import concourse.tile as tile
from concourse.bass_utils import run_bass_kernel_spmd

N_CORES = 8
B, C, H, W = 64, 3, 512, 512
BLOC = B // N_CORES  # batches per core
P = 128              # SBUF partitions / chunk height
NCH = H // P         # 128-row chunks per image
F32 = mybir.dt.float32
F32R = mybir.dt.float32r
GRAY_W = (0.299, 0.587, 0.114)

_NC = None          # cached compiled Bass module
LAST_RUN = None     # BassKernelResults of the most recent run (for test.py)


def _build_bass():
    nc = bacc.Bacc(
        "TRN2",
        target_bir_lowering=False,
        debug=False,
        num_devices=N_CORES,
    )
    x = nc.declare_dram_parameter("x", [BLOC, C, H, W], F32, isOutput=False)
    wts = nc.declare_dram_parameter("wts", [1, P, P], F32, isOutput=False)
    mvec = nc.declare_dram_parameter("mvec", [P, 1], F32, isOutput=False)
    out = nc.declare_dram_parameter("out", [BLOC, 1, H, W], F32, isOutput=True)

    # gray = GW[2] * (x0*(w0/w2) + x2  +  x1*(w1/w2)); the GW[2] scale is
    # folded into the mask tile on the host side. The two scaled terms are
    # computed on different engines in parallel, then summed on GpSimd.
    ga = GRAY_W[0] / GRAY_W[2]
    gb = GRAY_W[1] / GRAY_W[2]
    mult = mybir.AluOpType.mult
    add = mybir.AluOpType.add

    with tile.TileContext(nc) as tc:
        with (
            tc.tile_pool(name="consts", bufs=1) as consts,
            tc.tile_pool(name="xin", bufs=8) as xin,
            tc.tile_pool(name="work", bufs=6) as work,
            tc.tile_pool(name="psum", bufs=4, space="PSUM") as psum_pool,
        ):
            wdf = consts.tile([P, P], F32, tag="wdf")
            nc.sync.dma_start(wdf[:], wts[0])
            mv = consts.tile([P, 1], F32, tag="mvec")
            nc.sync.dma_start(mv[:], mvec[:])
            # one-time rounding of the stationary weight to f32r
            wd = consts.tile([P, P], F32R, tag="wd")
            nc.scalar.copy(wd[:], wdf[:])

            # out-DMA for chunk i is emitted at the top of iteration i+1 so
            # ACT's in-order stream never parks on the wait for DVE's final
            # transpose ahead of the next chunk's compute ops.
            pending = None
            for b in range(BLOC):
                for hc in range(NCH):
                    hs = hc * P
                    # one 768 KB DMA: channels side by side in the free dim
                    xt = xin.tile([P, C * W], F32, tag="x")
                    xsrc = x[b].rearrange("c (n p) w -> n p c w", p=P)[hc]
                    nc.sync.dma_start(
                        xt[:].rearrange("p (c w) -> p c w", w=W), xsrc
                    )
                    x0 = xt[:, 0 * W:1 * W]
                    x1 = xt[:, 1 * W:2 * W]
                    x2 = xt[:, 2 * W:3 * W]
                    # grayscale split across DVE / ACT / Pool so no engine saturates
                    g0 = work.tile([P, W], F32, tag="g0")
                    nc.vector.scalar_tensor_tensor(g0[:], x0, ga, x2, mult, add)
                    gs = work.tile([P, W], F32, tag="gs")
                    nc.scalar.mul(gs[:], x1, gb)
                    # delayed out-DMA sits after the gray mul in ACT's
                    # in-order stream: the mul's dep (input DMA) lands much
                    # earlier than the DMA's dep (prev chunk's transpose)
                    if pending is not None:
                        nc.scalar.dma_start(*pending)
                    g1 = work.tile([P, W], F32R, tag="g1")
                    nc.gpsimd.tensor_tensor(g1[:], gs[:], g0[:], add)
                    # H-direction DCT (f32r: full-rate single PE pass)
                    p1 = psum_pool.tile([P, W], F32, tag="p1")
                    nc.tensor.matmul(p1[:], wd[:], g1[:], start=True, stop=True)
                    # 32x32 block transpose straight out of PSUM
                    s1t = work.tile([P, W], F32, tag="s1t")
                    nc.vector.transpose(s1t[:], p1[:])
                    # W-direction DCT (plain f32: stream transpose cannot
                    # round its output to f32r)
                    p2 = psum_pool.tile([P, W], F32, tag="p2")
                    nc.tensor.matmul(p2[:], wdf[:], s1t[:], start=True, stop=True)
                    # high-pass mask + PSUM->SBUF move on ACT: columns with
                    # u<4 get a per-partition 0/1 scale (zero iff v<4), the
                    # u>=4 columns are a plain copy.
                    s2 = work.tile([P, W], F32, tag="s2")
                    p2v = p2[:].rearrange("p (g u) -> p g u", u=8)
                    s2v = s2[:].rearrange("p (g u) -> p g u", u=8)
                    nc.scalar.mul(s2v[:, :, 0:4], p2v[:, :, 0:4], mv[:])
                    nc.scalar.copy(s2v[:, :, 4:8], p2v[:, :, 4:8])
                    # block transpose back to natural layout
                    s2t = work.tile([P, W], F32, tag="s2t", bufs=8)
                    nc.vector.transpose(s2t[:], s2[:])
                    # outputs ride the ACT HWDGE queue; inputs own the SP queue
                    pending = (out[b, 0, hs:hs + P, :], s2t[:])
            nc.scalar.dma_start(*pending)
    nc.compile()
    return nc


def _host_constants(dct_matrix, mask):
    D = np.asarray(dct_matrix, dtype=np.float32)
    M = np.asarray(mask, dtype=np.float32)
    dctT = np.kron(np.eye(P // 8, dtype=np.float32), D.T).astype(np.float32)
    # fold the trailing grayscale scale (GRAY_W[2]) into the (shared) DCT
    # weight as sqrt(c): both matmuls apply it, so the chain gains c total.
    wts = (np.sqrt(np.float32(GRAY_W[2])) * dctT).astype(np.float32)[None]
    # per-partition mask column for the u<4 output columns: M[u<4, v] is
    # constant in u there, so it reduces to a v-indexed 0/1 vector.
    pi = np.arange(P)
    mvec = np.ascontiguousarray(M[0, pi % 8], dtype=np.float32).reshape(P, 1)
    return wts, mvec


def kernel(x, dct_matrix, mask):
    global _NC, LAST_RUN
    x = np.ascontiguousarray(np.asarray(x, dtype=np.float32))
    assert x.shape == (B, C, H, W)
    wts, mvec = _host_constants(dct_matrix, mask)

    if _NC is None:
        _NC = _build_bass()

    in_maps = [
        {"x": np.ascontiguousarray(x[i * BLOC:(i + 1) * BLOC]),
         "wts": wts, "mvec": mvec}
        for i in range(N_CORES)
    ]
    trace = bool(int(os.environ.get("DCT_TRACE", "0")))
    LAST_RUN = run_bass_kernel_spmd(
        _NC, in_maps, list(range(N_CORES)), trace=trace,
    )
    out = np.concatenate([LAST_RUN.results[i]["out"] for i in range(N_CORES)], axis=0)
    return out


# revision 11
# speedup vs baseline: 1.0278x; 1.0278x over previous
"""Trainium2 Bass kernel for blocked-DCT high-frequency extractor.

Computes, for x (64, 3, 512, 512) f32:
  gray = 0.299*R + 0.587*G + 0.114*B                     (B,1,H,W)
  per 8x8 block:  Y = mask * (D @ block @ D.T)           (2D DCT + high-pass)
  output (64, 1, 512, 512) f32

Strategy: pure data parallel over batch (8 batches/core on 8 cores). The
kernel is HBM-bound: 24 MiB in + 8 MiB out per core => ~94 us floor at the
~358 GB/s per-core HBM limit. All compute engines are kept below the DMA
stream rate (~2.74 us per 128-row chunk) and the per-chunk work is
SOFTWARE-PIPELINED across three stages so the in-order engine streams never
serialize on the cross-engine dependency chain:

  stage A (chunk k):   in-DMA -> gray (DVE stt / ACT mul / GpSimd add,
                       rounded to f32r) -> H-DCT matmul (f32r, 1 cyc/row)
  stage B (chunk k-1): ACT PSUM->SBUF cast to bf16 -> DVE 32x32 block
                       transpose (bf16) -> W-DCT matmul (bf16)
  stage C (chunk k-2): high-pass mask on ACT (two strided PSUM->SBUF
                       copies; u<4 columns scaled by a per-partition 0/1
                       vector) -> DVE block transpose back (f32)
  stage D (chunk k-3): 256 KB contiguous output DMA on the ACT HWDGE
                       queue (inputs own the SP queue)

The 32x32 block transpose is an involution whose block-nesting (8 | 32)
makes both DCT matmuls use the same I_16 kron D^T stationary weight (one
f32r copy, one bf16 copy) and lands the final result in natural row-major
layout with zero TensorE transposes.
"""

import os

import numpy as np

import concourse.bacc as bacc
import concourse.mybir as mybir
import concourse.tile as tile
from concourse.bass_utils import run_bass_kernel_spmd

N_CORES = 8
B, C, H, W = 64, 3, 512, 512
BLOC = B // N_CORES  # batches per core
P = 128              # SBUF partitions / chunk height
NCH = H // P         # 128-row chunks per image
TOT = BLOC * NCH     # chunks per core
F32 = mybir.dt.float32
F32R = mybir.dt.float32r
BF16 = mybir.dt.bfloat16
GRAY_W = (0.299, 0.587, 0.114)

_NC = None          # cached compiled Bass module
LAST_RUN = None     # BassKernelResults of the most recent run (for test.py)


def _build_bass():
    nc = bacc.Bacc(
        "TRN2",
        target_bir_lowering=False,
        debug=False,
        num_devices=N_CORES,
    )
    x = nc.declare_dram_parameter("x", [BLOC, C, H, W], F32, isOutput=False)
    wts = nc.declare_dram_parameter("wts", [1, P, P], F32, isOutput=False)
    mvec = nc.declare_dram_parameter("mvec", [P, 1], F32, isOutput=False)
    out = nc.declare_dram_parameter("out", [BLOC, 1, H, W], F32, isOutput=True)

    ga = GRAY_W[0] / GRAY_W[2]
    gb = GRAY_W[1] / GRAY_W[2]
    mult = mybir.AluOpType.mult
    add = mybir.AluOpType.add

    with tile.TileContext(nc) as tc:
        with (
            tc.tile_pool(name="consts", bufs=1) as consts,
            tc.tile_pool(name="xin", bufs=8) as xin,
            tc.tile_pool(name="work", bufs=3) as work,
            tc.tile_pool(name="psum", bufs=3, space="PSUM") as psum_pool,
        ):
            wdf = consts.tile([P, P], F32, tag="wdf")
            nc.sync.dma_start(wdf[:], wts[0])
            mv = consts.tile([P, 1], F32, tag="mvec")
            nc.sync.dma_start(mv[:], mvec[:])
            # stationary weight in f32r (H-DCT) and bf16 (W-DCT)
            wd = consts.tile([P, P], F32R, tag="wd")
            nc.scalar.copy(wd[:], wdf[:])
            wdb = consts.tile([P, P], BF16, tag="wdb")
            nc.scalar.copy(wdb[:], wdf[:])

            st = [dict() for _ in range(TOT)]

            def stage_a(k):
                b, hc = divmod(k, NCH)
                # one 768 KB DMA: channels side by side in the free dim
                xt = xin.tile([P, C * W], F32, tag="x")
                xsrc = x[b].rearrange("c (n p) w -> n p c w", p=P)[hc]
                nc.sync.dma_start(
                    xt[:].rearrange("p (c w) -> p c w", w=W), xsrc
                )
                x0 = xt[:, 0 * W:1 * W]
                x1 = xt[:, 1 * W:2 * W]
                x2 = xt[:, 2 * W:3 * W]
                # grayscale split across DVE / ACT / Pool
                g0 = work.tile([P, W], F32, tag="g0")
                nc.vector.scalar_tensor_tensor(g0[:], x0, ga, x2, mult, add)
                gs = work.tile([P, W], F32, tag="gs")
                nc.scalar.mul(gs[:], x1, gb)
                g1 = work.tile([P, W], F32R, tag="g1")
                nc.gpsimd.tensor_tensor(g1[:], gs[:], g0[:], add)
                # H-direction DCT: f32r runs a single full-rate PE pass
                p1 = psum_pool.tile([P, W], F32, tag="p1")
                nc.tensor.matmul(p1[:], wd[:], g1[:], start=True, stop=True)
                st[k]["p1"] = p1

            def stage_b(k):
                p1 = st[k].pop("p1")
                # PSUM->SBUF move with bf16 cast on ACT (stream transpose
                # cannot change dtypes), then 32x32 block transpose on DVE
                s1c = work.tile([P, W], BF16, tag="s1c")
                nc.scalar.copy(s1c[:], p1[:])
                s1t = work.tile([P, W], BF16, tag="s1t")
                nc.vector.transpose(s1t[:], s1c[:])
                # W-direction DCT in bf16
                p2 = psum_pool.tile([P, W], F32, tag="p2")
                nc.tensor.matmul(p2[:], wdb[:], s1t[:], start=True, stop=True)
                st[k]["p2"] = p2

            def stage_c(k):
                p2 = st[k].pop("p2")
                # high-pass mask + PSUM->SBUF move on ACT: columns with u<4
                # get a per-partition 0/1 scale (zero iff v<4), u>=4 columns
                # are a plain copy.
                s2 = work.tile([P, W], F32, tag="s2")
                p2v = p2[:].rearrange("p (g u) -> p g u", u=8)
                s2v = s2[:].rearrange("p (g u) -> p g u", u=8)
                nc.scalar.mul(s2v[:, :, 0:4], p2v[:, :, 0:4], mv[:])
                nc.scalar.copy(s2v[:, :, 4:8], p2v[:, :, 4:8])
                # block transpose back to natural layout
                s2t = work.tile([P, W], F32, tag="s2t", bufs=6)
                nc.vector.transpose(s2t[:], s2[:])
                st[k]["s2t"] = s2t

            def stage_d(k):
                b, hc = divmod(k, NCH)
                hs = hc * P
                s2t = st[k].pop("s2t")
                # outputs ride the ACT HWDGE queue; by lag 3 the transpose
                # has long completed, so ACT never parks on this wait
                nc.scalar.dma_start(out[b, 0, hs:hs + P, :], s2t[:])

            for it in range(TOT + 3):
                if it < TOT:
                    stage_a(it)
                if 1 <= it <= TOT:
                    stage_b(it - 1)
                if 2 <= it <= TOT + 1:
                    stage_c(it - 2)
                if 3 <= it:
                    stage_d(it - 3)
    nc.compile()
    return nc


def _host_constants(dct_matrix, mask):
    D = np.asarray(dct_matrix, dtype=np.float32)
    M = np.asarray(mask, dtype=np.float32)
    dctT = np.kron(np.eye(P // 8, dtype=np.float32), D.T).astype(np.float32)
    # fold the trailing grayscale scale (GRAY_W[2]) into the (shared) DCT
    # weight as sqrt(c): both matmuls apply it, so the chain gains c total.
    wts = (np.sqrt(np.float32(GRAY_W[2])) * dctT).astype(np.float32)[None]
    # per-partition mask column for the u<4 output columns: M[u<4, v] is
    # constant in u there, so it reduces to a v-indexed 0/1 vector.
    pi = np.arange(P)
    mvec = np.ascontiguousarray(M[0, pi % 8], dtype=np.float32).reshape(P, 1)
    return wts, mvec


def kernel(x, dct_matrix, mask):
    global _NC, LAST_RUN
    x = np.ascontiguousarray(np.asarray(x, dtype=np.float32))
    assert x.shape == (B, C, H, W)
    wts, mvec = _host_constants(dct_matrix, mask)

    if _NC is None:
        _NC = _build_bass()

    in_maps = [
        {"x": np.ascontiguousarray(x[i * BLOC:(i + 1) * BLOC]),
         "wts": wts, "mvec": mvec}
        for i in range(N_CORES)
    ]
    trace = bool(int(os.environ.get("DCT_TRACE", "0")))
    LAST_RUN = run_bass_kernel_spmd(
        _NC, in_maps, list(range(N_CORES)), trace=trace,
    )
    out = np.concatenate([LAST_RUN.results[i]["out"] for i in range(N_CORES)], axis=0)
    return out


# revision 18
# speedup vs baseline: 1.1219x; 1.0915x over previous
"""Trainium2 Bass kernel for blocked-DCT high-frequency extractor.

Computes, for x (64, 3, 512, 512) f32:
  gray = 0.299*R + 0.587*G + 0.114*B                     (B,1,H,W)
  per 8x8 block:  Y = mask * (D @ block @ D.T)           (2D DCT + high-pass)
  output (64, 1, 512, 512) f32

Strategy: pure data parallel over batch (8 batches/core on 8 cores). The
kernel is HBM-bound: 24 MiB in + 8 MiB out per core => ~94 us floor at the
~358 GB/s per-core HBM limit. All compute engines are kept below the DMA
stream rate (~2.74 us per 128-row chunk) and the per-chunk work is
SOFTWARE-PIPELINED across three stages so the in-order engine streams never
serialize on the cross-engine dependency chain:

  stage A (chunk k):   in-DMA -> gray (DVE stt / ACT mul / GpSimd add,
                       rounded to f32r) -> H-DCT matmul (f32r, 1 cyc/row)
  stage B (chunk k-1): ACT PSUM->SBUF cast to bf16 -> DVE 32x32 block
                       transpose (bf16) -> W-DCT matmul (bf16)
  stage C (chunk k-2): high-pass mask on ACT (two strided PSUM->SBUF
                       copies; u<4 columns scaled by a per-partition 0/1
                       vector) -> DVE block transpose back (f32)
  stage D (chunk k-3): 256 KB contiguous output DMA on the ACT HWDGE
                       queue (inputs own the SP queue)

The 32x32 block transpose is an involution whose block-nesting (8 | 32)
makes both DCT matmuls use the same I_16 kron D^T stationary weight (one
f32r copy, one bf16 copy) and lands the final result in natural row-major
layout with zero TensorE transposes.
"""

import os

import numpy as np

import concourse.bacc as bacc
import concourse.mybir as mybir
import concourse.tile as tile
from concourse.bass_utils import run_bass_kernel_spmd

N_CORES = 8
B, C, H, W = 64, 3, 512, 512
BLOC = B // N_CORES  # batches per core
P = 128              # SBUF partitions / chunk height
NCH = H // P         # 128-row chunks per image
TOT = BLOC * NCH     # chunks per core
F32 = mybir.dt.float32
F32R = mybir.dt.float32r
BF16 = mybir.dt.bfloat16
GRAY_W = (0.299, 0.587, 0.114)

_NC = None          # cached compiled Bass module
LAST_RUN = None     # BassKernelResults of the most recent run (for test.py)


def _build_bass():
    nc = bacc.Bacc(
        "TRN2",
        target_bir_lowering=False,
        debug=False,
        num_devices=N_CORES,
    )
    x = nc.declare_dram_parameter("x", [BLOC, C, H, W], F32, isOutput=False)
    wts = nc.declare_dram_parameter("wts", [1, P, 2 * P], F32, isOutput=False)
    out = nc.declare_dram_parameter("out", [BLOC, 1, H, W], F32, isOutput=True)

    ga = GRAY_W[0] / GRAY_W[2]
    gb = GRAY_W[1] / GRAY_W[2]
    mult = mybir.AluOpType.mult
    add = mybir.AluOpType.add

    with tile.TileContext(nc) as tc:
        with (
            tc.tile_pool(name="consts", bufs=1) as consts,
            tc.tile_pool(name="xin", bufs=8) as xin,
            tc.tile_pool(name="work", bufs=3) as work,
            tc.tile_pool(name="psum", bufs=3, space="PSUM") as psum_pool,
        ):
            wdf = consts.tile([P, 2 * P], F32, tag="wdf")
            nc.sync.dma_start(wdf[:], wts[0])
            # stationary weights: f32r for the H-DCT; bf16 plain + bf16
            # row-masked (v<4 zeroed) for the two W-DCT column groups
            wd = consts.tile([P, P], F32R, tag="wd")
            nc.scalar.copy(wd[:], wdf[:, 0:P])
            wdb = consts.tile([P, P], BF16, tag="wdb")
            nc.scalar.copy(wdb[:], wdf[:, 0:P])
            wdm = consts.tile([P, P], BF16, tag="wdm")
            nc.scalar.copy(wdm[:], wdf[:, P:2 * P])

            st = [dict() for _ in range(TOT)]

            def stage_a(k):
                b, hc = divmod(k, NCH)
                # one 768 KB DMA: channels side by side in the free dim
                xt = xin.tile([P, C * W], F32, tag="x")
                xsrc = x[b].rearrange("c (n p) w -> n p c w", p=P)[hc]
                nc.sync.dma_start(
                    xt[:].rearrange("p (c w) -> p c w", w=W), xsrc
                )
                x0 = xt[:, 0 * W:1 * W]
                x1 = xt[:, 1 * W:2 * W]
                x2 = xt[:, 2 * W:3 * W]
                # grayscale split across DVE / ACT / Pool
                g0 = work.tile([P, W], F32, tag="g0")
                nc.vector.scalar_tensor_tensor(g0[:], x0, ga, x2, mult, add)
                gs = work.tile([P, W], F32, tag="gs")
                nc.scalar.mul(gs[:], x1, gb)
                g1 = work.tile([P, W], F32R, tag="g1")
                nc.gpsimd.tensor_tensor(g1[:], gs[:], g0[:], add)
                # H-direction DCT: f32r runs a single full-rate PE pass
                p1 = psum_pool.tile([P, W], F32, tag="p1")
                nc.tensor.matmul(p1[:], wd[:], g1[:], start=True, stop=True)
                st[k]["p1"] = p1

            def stage_b(k):
                p1 = st[k].pop("p1")
                # PSUM->SBUF move with bf16 cast on ACT (stream transpose
                # cannot change dtypes), then 32x32 block transpose on DVE
                s1c = work.tile([P, W], BF16, tag="s1c")
                nc.scalar.copy(s1c[:], p1[:])
                s1t = work.tile([P, W], BF16, tag="s1t")
                nc.vector.transpose(s1t[:], s1c[:])
                # W-direction DCT in bf16, split into the two mask column
                # groups: u<4 columns use the v<4-row-zeroed stationary, so
                # the high-pass mask is free
                p2 = psum_pool.tile([P, W], F32, tag="p2")
                s1v = s1t[:].rearrange("p (g u) -> p g u", u=8)
                p2v = p2[:].rearrange("p (g u) -> p g u", u=8)
                nc.tensor.matmul(
                    p2v[:, :, 0:4], wdm[:], s1v[:, :, 0:4],
                    start=True, stop=True,
                )
                nc.tensor.matmul(
                    p2v[:, :, 4:8], wdb[:], s1v[:, :, 4:8],
                    start=True, stop=True,
                )
                st[k]["p2"] = p2

            def stage_c(k):
                p2 = st[k].pop("p2")
                # plain PSUM->SBUF move on ACT (mask already applied in MM2)
                s2 = work.tile([P, W], F32, tag="s2")
                nc.scalar.copy(s2[:], p2[:])
                # block transpose back to natural layout
                s2t = work.tile([P, W], F32, tag="s2t", bufs=6)
                nc.vector.transpose(s2t[:], s2[:])
                st[k]["s2t"] = s2t

            def stage_d(k):
                b, hc = divmod(k, NCH)
                hs = hc * P
                s2t = st[k].pop("s2t")
                # outputs ride the ACT HWDGE queue; by lag 3 the transpose
                # has long completed, so ACT never parks on this wait
                nc.scalar.dma_start(out[b, 0, hs:hs + P, :], s2t[:])

            for it in range(TOT + 3):
                if it < TOT:
                    stage_a(it)
                if 1 <= it <= TOT:
                    stage_b(it - 1)
                if 2 <= it <= TOT + 1:
                    stage_c(it - 2)
                if 3 <= it:
                    stage_d(it - 3)
    nc.compile()
    return nc


def _host_constants(dct_matrix, mask):
    D = np.asarray(dct_matrix, dtype=np.float32)
    M = np.asarray(mask, dtype=np.float32)
    dctT = np.kron(np.eye(P // 8, dtype=np.float32), D.T).astype(np.float32)
    # fold the trailing grayscale scale (GRAY_W[2]) into the (shared) DCT
    # weight as sqrt(c): both matmuls apply it, so the chain gains c total.
    base = (np.sqrt(np.float32(GRAY_W[2])) * dctT).astype(np.float32)
    # masked variant for the u<4 moving-column group of MM2: zero the
    # stationary's free columns i with i%8 < 4 (those produce the v<4
    # output partitions, which the high-pass mask zeroes when u<4).
    colmask = (np.arange(P) % 8 >= 4).astype(np.float32)[None, :]
    # M[u<4, v<4] == 0 in the reference; sanity-anchor the derivation
    assert M[0, 0] == 0.0 and M[0, 4] == 1.0 and M[4, 0] == 1.0
    wts = np.concatenate([base, base * colmask], axis=1)[None]
    return wts


def kernel(x, dct_matrix, mask):
    global _NC, LAST_RUN
    x = np.ascontiguousarray(np.asarray(x, dtype=np.float32))
    assert x.shape == (B, C, H, W)
    wts = _host_constants(dct_matrix, mask)

    if _NC is None:
        _NC = _build_bass()

    in_maps = [
        {"x": np.ascontiguousarray(x[i * BLOC:(i + 1) * BLOC]), "wts": wts}
        for i in range(N_CORES)
    ]
    trace = bool(int(os.environ.get("DCT_TRACE", "0")))
    LAST_RUN = run_bass_kernel_spmd(
        _NC, in_maps, list(range(N_CORES)), trace=trace,
    )
    out = np.concatenate([LAST_RUN.results[i]["out"] for i in range(N_CORES)], axis=0)
    return out
